# revision 1
# baseline (speedup 1.0000x reference)
"""v5: strided-rhs implicit-GEMM conv, all-sync input DMAs, t-outer loop.

vs v2: x DMAs move scalar->sync (HWDGE, fast) with xin bufs=1 so image
1-3 prefetches are gated by slot reuse and cannot inflate the first
matmul's DMA wait; loop order is t-outer/h-inner so the xa slot releases
~23% into each image, giving the next image's prefetch a wide margin.
"""

import sys

if "/opt/trn_rl_repo" not in sys.path:
    sys.path.insert(0, "/opt/trn_rl_repo")

import numpy as np

N, C_IN, H, W = 32, 128, 56, 56
C_OUT, KH, KW = 256, 3, 3
N_CORES = 8
IMGS = N // N_CORES
HP, WP = H + 2, W + 2
RPT = 8
NT = H // RPT          # 7
TF = RPT * W           # 448
NH = C_OUT // 128      # 2

XA_R0, XA_R1 = 0, 26   # rows for tiles t=0..2
XB_R0, XB_R1 = 24, 58  # rows for tiles t=3..6
T_SPLIT = 3
OUT_SPLIT = 4          # chunk A = tiles 0..3
N_WARMUP_MM = 18

_CACHE = {}


def _build_program():
    import concourse.mybir as mybir
    import concourse.tile as tile
    from concourse import bacc

    F32 = mybir.dt.float32
    F32R = mybir.dt.float32r

    nc = bacc.Bacc("TRN2", target_bir_lowering=False, debug=False,
                   enable_asserts=False)

    xp = nc.dram_tensor("xp", [IMGS, C_IN, HP, WP], F32R,
                        kind="ExternalInput").ap()
    w = nc.dram_tensor("w", [C_IN, KH * KW, C_OUT], F32R,
                       kind="ExternalInput").ap()
    b = nc.dram_tensor("b", [128, NH], F32, kind="ExternalInput").ap()
    out = nc.dram_tensor("out", [IMGS, C_OUT, H, W], F32,
                         kind="ExternalOutput").ap()
    out_v = out.rearrange("n c a b -> n c (a b)")

    with tile.TileContext(nc) as tc:
        with (
            tc.tile_pool(name="consts", bufs=1) as consts,
            tc.tile_pool(name="xin", bufs=1) as xin,
            tc.tile_pool(name="outp", bufs=2) as outp,
            tc.tile_pool(name="psum", bufs=7, space="PSUM") as psum,
        ):
            scratch = consts.tile([128, TF], F32, tag="scratch")
            nc.gpsimd.memset(scratch[:], 0.0)
            scr_r = scratch[:, :].bitcast(F32R)
            warm_ps = psum.tile([128, TF], F32, tag="warm", bufs=1)
            for _ in range(N_WARMUP_MM):
                nc.tensor.matmul(warm_ps[:, :], lhsT=scr_r[:, :128],
                                 rhs=scr_r[:, :], start=True, stop=True)

            xts = {}
            for img in range(IMGS):
                xa = xin.tile([C_IN, XA_R1 - XA_R0, WP], F32R, tag="xa")
                nc.sync.dma_start(out=xa[:], in_=xp[img, :, XA_R0:XA_R1])
                if img == 0:
                    # w directly after xa0: the first matmul's cumulative
                    # DMA-sem wait then covers exactly [xa0, w]
                    w_sb = consts.tile([C_IN, KH * KW, C_OUT], F32R, tag="w")
                    nc.sync.dma_start(out=w_sb[:], in_=w)
                xb = xin.tile([C_IN, XB_R1 - XB_R0, WP], F32R, tag="xb")
                nc.sync.dma_start(out=xb[:], in_=xp[img, :, XB_R0:XB_R1])
                xts[img] = (xa, xb)
                if img == 0:
                    b_sb = consts.tile([128, NH], F32, tag="b")
                    nc.sync.dma_start(out=b_sb[:], in_=b)

            for img in range(IMGS):
                xa, xb = xts[img]
                ots = [outp.tile([128, H * W], F32, tag=f"ot{h}",
                                 name=f"ot{img}_{h}")
                       for h in range(NH)]
                for t in range(NT):
                    if t < T_SPLIT:
                        src, r_off = xa, XA_R0
                    else:
                        src, r_off = xb, XB_R0
                    r0 = RPT * t - r_off
                    for h in range(NH):
                        pt = psum.tile([128, TF], F32, tag="pt")
                        for k in range(KH * KW):
                            kh, kw = divmod(k, KW)
                            nc.tensor.matmul(
                                pt[:, :],
                                lhsT=w_sb[:, k, h * 128:(h + 1) * 128],
                                rhs=src[:, r0 + kh:r0 + kh + RPT, kw:kw + W],
                                start=(k == 0),
                                stop=(k == KH * KW - 1),
                            )
                        nc.vector.tensor_scalar_add(
                            out=ots[h][:, t * TF:(t + 1) * TF],
                            in0=pt[:, :],
                            scalar1=b_sb[:, h:h + 1],
                        )
                        nc.sync.dma_start(
                            out=out_v[img, h * 128:(h + 1) * 128,
                                      t * TF:(t + 1) * TF],
                            in_=ots[h][:, t * TF:(t + 1) * TF])
    nc.compile()
    return nc


def get_program():
    if "nc" not in _CACHE:
        _CACHE["nc"] = _build_program()
    return _CACHE["nc"]


def make_in_maps(x, weight, bias):
    x = np.asarray(x, dtype=np.float32)
    weight = np.asarray(weight, dtype=np.float32)
    bias = np.asarray(bias, dtype=np.float32)

    xpad = np.zeros((N, C_IN, HP, WP), dtype=np.float32)
    xpad[:, :, 1:1 + H, 1:1 + W] = x
    w_t = np.ascontiguousarray(
        weight.transpose(1, 2, 3, 0).reshape(C_IN, KH * KW, C_OUT))
    b2 = np.ascontiguousarray(bias.reshape(NH, 128).T)

    return [
        {
            "xp": np.ascontiguousarray(xpad[i * IMGS:(i + 1) * IMGS]),
            "w": w_t,
            "b": b2,
        }
        for i in range(N_CORES)
    ]


def kernel(x, weight, bias):
    from concourse.bass_utils import run_bass_kernel_spmd

    nc = get_program()
    in_maps = make_in_maps(x, weight, bias)
    res = run_bass_kernel_spmd(nc, in_maps, core_ids=list(range(N_CORES)))
    return np.concatenate([res.results[i]["out"] for i in range(N_CORES)],
                          axis=0)



# revision 2
# speedup vs baseline: 1.0689x; 1.0689x over previous
"""v6: bf16 implicit-GEMM conv (v5 structure, bf16 x/w operands).

vs v5: x and w cast to bf16 on host. Halves input DMA bytes (head
shrinks ~3us) and halves PE operand energy (fp32_mode=HIGH off) to
probe whether the 87.4% power-throttle on the matmul block lifts.
PSUM stays f32; bias-add and output stay f32, so only input
quantization error (~4e-3 rel) is added, vs 2e-2 tolerance.
"""

import sys

if "/opt/trn_rl_repo" not in sys.path:
    sys.path.insert(0, "/opt/trn_rl_repo")

import ml_dtypes
import numpy as np

N, C_IN, H, W = 32, 128, 56, 56
C_OUT, KH, KW = 256, 3, 3
N_CORES = 8
IMGS = N // N_CORES
HP, WP = H + 2, W + 2
RPT = 8
NT = H // RPT          # 7
TF = RPT * W           # 448
NH = C_OUT // 128      # 2

XA_R0, XA_R1 = 0, 26   # rows for tiles t=0..2
XB_R0, XB_R1 = 24, 58  # rows for tiles t=3..6
T_SPLIT = 3
N_WARMUP_MM = 14

_CACHE = {}


def _build_program():
    import concourse.mybir as mybir
    import concourse.tile as tile
    from concourse import bacc

    F32 = mybir.dt.float32
    BF16 = mybir.dt.bfloat16

    nc = bacc.Bacc("TRN2", target_bir_lowering=False, debug=False,
                   enable_asserts=False)

    xp = nc.dram_tensor("xp", [IMGS, C_IN, HP, WP], BF16,
                        kind="ExternalInput").ap()
    w = nc.dram_tensor("w", [C_IN, KH * KW, C_OUT], BF16,
                       kind="ExternalInput").ap()
    b = nc.dram_tensor("b", [128, NH], F32, kind="ExternalInput").ap()
    out = nc.dram_tensor("out", [IMGS, C_OUT, H, W], F32,
                         kind="ExternalOutput").ap()
    out_v = out.rearrange("n c a b -> n c (a b)")

    with tile.TileContext(nc) as tc:
        with (
            tc.tile_pool(name="consts", bufs=1) as consts,
            tc.tile_pool(name="xin", bufs=1) as xin,
            tc.tile_pool(name="outp", bufs=2) as outp,
            tc.tile_pool(name="psum", bufs=7, space="PSUM") as psum,
        ):
            scratch = consts.tile([128, TF], BF16, tag="scratch")
            nc.gpsimd.memset(scratch[:], 0.0)
            warm_ps = psum.tile([128, TF], F32, tag="warm", bufs=1)
            for _ in range(N_WARMUP_MM):
                nc.tensor.matmul(warm_ps[:, :], lhsT=scratch[:, :128],
                                 rhs=scratch[:, :], start=True, stop=True)

            xts = {}
            for img in range(IMGS):
                xa = xin.tile([C_IN, XA_R1 - XA_R0, WP], BF16, tag="xa")
                nc.sync.dma_start(out=xa[:], in_=xp[img, :, XA_R0:XA_R1])
                if img == 0:
                    # w directly after xa0: the first matmul's cumulative
                    # DMA-sem wait then covers exactly [xa0, w]
                    w_sb = consts.tile([C_IN, KH * KW, C_OUT], BF16, tag="w")
                    nc.sync.dma_start(out=w_sb[:], in_=w)
                xb = xin.tile([C_IN, XB_R1 - XB_R0, WP], BF16, tag="xb")
                nc.sync.dma_start(out=xb[:], in_=xp[img, :, XB_R0:XB_R1])
                xts[img] = (xa, xb)
                if img == 0:
                    b_sb = consts.tile([128, NH], F32, tag="b")
                    nc.sync.dma_start(out=b_sb[:], in_=b)

            for img in range(IMGS):
                xa, xb = xts[img]
                ots = [outp.tile([128, H * W], F32, tag=f"ot{h}",
                                 name=f"ot{img}_{h}")
                       for h in range(NH)]
                for t in range(NT):
                    if t < T_SPLIT:
                        src, r_off = xa, XA_R0
                    else:
                        src, r_off = xb, XB_R0
                    r0 = RPT * t - r_off
                    for h in range(NH):
                        pt = psum.tile([128, TF], F32, tag="pt")
                        for k in range(KH * KW):
                            kh, kw = divmod(k, KW)
                            nc.tensor.matmul(
                                pt[:, :],
                                lhsT=w_sb[:, k, h * 128:(h + 1) * 128],
                                rhs=src[:, r0 + kh:r0 + kh + RPT, kw:kw + W],
                                start=(k == 0),
                                stop=(k == KH * KW - 1),
                            )
                        nc.vector.tensor_scalar_add(
                            out=ots[h][:, t * TF:(t + 1) * TF],
                            in0=pt[:, :],
                            scalar1=b_sb[:, h:h + 1],
                        )
                        nc.sync.dma_start(
                            out=out_v[img, h * 128:(h + 1) * 128,
                                      t * TF:(t + 1) * TF],
                            in_=ots[h][:, t * TF:(t + 1) * TF])
    nc.compile()
    return nc


def get_program():
    if "nc" not in _CACHE:
        _CACHE["nc"] = _build_program()
    return _CACHE["nc"]


def make_in_maps(x, weight, bias):
    x = np.asarray(x, dtype=np.float32)
    weight = np.asarray(weight, dtype=np.float32)
    bias = np.asarray(bias, dtype=np.float32)

    xpad = np.zeros((N, C_IN, HP, WP), dtype=ml_dtypes.bfloat16)
    xpad[:, :, 1:1 + H, 1:1 + W] = x.astype(ml_dtypes.bfloat16)
    w_t = np.ascontiguousarray(
        weight.transpose(1, 2, 3, 0).reshape(C_IN, KH * KW, C_OUT)
    ).astype(ml_dtypes.bfloat16)
    b2 = np.ascontiguousarray(bias.reshape(NH, 128).T)

    return [
        {
            "xp": np.ascontiguousarray(xpad[i * IMGS:(i + 1) * IMGS]),
            "w": w_t,
            "b": b2,
        }
        for i in range(N_CORES)
    ]


def kernel(x, weight, bias):
    from concourse.bass_utils import run_bass_kernel_spmd

    nc = get_program()
    in_maps = make_in_maps(x, weight, bias)
    res = run_bass_kernel_spmd(nc, in_maps, core_ids=list(range(N_CORES)))
    return np.concatenate([res.results[i]["out"] for i in range(N_CORES)],
                          axis=0)


# revision 3
# speedup vs baseline: 1.1171x; 1.0451x over previous
"""v7: bf16 conv, two HW DMA queues, early w load, split tail tile.

vs v6: the head stall was the w DMA sharing the sync HWDGE queue with
xa0/xb0/b — round-robin packet interleave stretched w's completion
(and its inline 4B semaphore increments) from 10.4us to 15.4us. Now:
sync queue = w (first trigger) + output tiles; scalar queue = xa/b/xb
inputs. Scratch memset moves to DVE so warmup starts at body entry,
warmup trimmed 14->12, and the final (img3,t6,h1) tile computes as
two 224-col halves so the tail drain after the last matmul is ~half.
"""

import sys

if "/opt/trn_rl_repo" not in sys.path:
    sys.path.insert(0, "/opt/trn_rl_repo")

import ml_dtypes
import numpy as np

N, C_IN, H, W = 32, 128, 56, 56
C_OUT, KH, KW = 256, 3, 3
N_CORES = 8
IMGS = N // N_CORES
HP, WP = H + 2, W + 2
RPT = 8
NT = H // RPT          # 7
TF = RPT * W           # 448
NH = C_OUT // 128      # 2

XA_R0, XA_R1 = 0, 26   # rows for tiles t=0..2
XB_R0, XB_R1 = 24, 58  # rows for tiles t=3..6
T_SPLIT = 3
N_WARMUP_MM = 12

_CACHE = {}


def _build_program():
    import concourse.mybir as mybir
    import concourse.tile as tile
    from concourse import bacc

    F32 = mybir.dt.float32
    BF16 = mybir.dt.bfloat16

    nc = bacc.Bacc("TRN2", target_bir_lowering=False, debug=False,
                   enable_asserts=False)

    xp = nc.dram_tensor("xp", [IMGS, C_IN, HP, WP], BF16,
                        kind="ExternalInput").ap()
    w = nc.dram_tensor("w", [C_IN, KH * KW, C_OUT], BF16,
                       kind="ExternalInput").ap()
    b = nc.dram_tensor("b", [128, NH], F32, kind="ExternalInput").ap()
    out = nc.dram_tensor("out", [IMGS, C_OUT, H, W], F32,
                         kind="ExternalOutput").ap()
    out_v = out.rearrange("n c a b -> n c (a b)")

    with tile.TileContext(nc) as tc:
        with (
            tc.tile_pool(name="consts", bufs=1) as consts,
            tc.tile_pool(name="xin", bufs=1) as xin,
            tc.tile_pool(name="outp", bufs=2) as outp,
            tc.tile_pool(name="psum", bufs=7, space="PSUM") as psum,
        ):
            scratch = consts.tile([128, TF], BF16, tag="scratch")
            nc.vector.memset(scratch[:], 0.0)

            # w is the sync queue's only head-time transfer: full bandwidth
            w_sb = consts.tile([C_IN, KH * KW, C_OUT], BF16, tag="w")
            nc.sync.dma_start(out=w_sb[:], in_=w)

            b_sb = consts.tile([128, NH], F32, tag="b")
            xts = {}
            for img in range(IMGS):
                xa = xin.tile([C_IN, XA_R1 - XA_R0, WP], BF16, tag="xa")
                nc.scalar.dma_start(out=xa[:], in_=xp[img, :, XA_R0:XA_R1])
                if img == 0:
                    nc.scalar.dma_start(out=b_sb[:], in_=b)
                xb = xin.tile([C_IN, XB_R1 - XB_R0, WP], BF16, tag="xb")
                nc.scalar.dma_start(out=xb[:], in_=xp[img, :, XB_R0:XB_R1])
                xts[img] = (xa, xb)

            warm_ps = psum.tile([128, TF], F32, tag="warm", bufs=1)
            for _ in range(N_WARMUP_MM):
                nc.tensor.matmul(warm_ps[:, :], lhsT=scratch[:, :128],
                                 rhs=scratch[:, :], start=True, stop=True)

            def conv_tile(src, r0, kcol0, rows, pt_cols):
                """9-matmul accumulation for `rows` output H-rows."""
                pt = psum.tile([128, pt_cols], F32, tag="pt")
                for k in range(KH * KW):
                    kh, kw = divmod(k, KW)
                    nc.tensor.matmul(
                        pt[:, :rows * W],
                        lhsT=w_sb[:, k, kcol0:kcol0 + 128],
                        rhs=src[:, r0 + kh:r0 + kh + rows, kw:kw + W],
                        start=(k == 0),
                        stop=(k == KH * KW - 1),
                    )
                return pt

            for img in range(IMGS):
                xa, xb = xts[img]
                ots = [outp.tile([128, H * W], F32, tag=f"ot{h}",
                                 name=f"ot{img}_{h}")
                       for h in range(NH)]
                for t in range(NT):
                    if t < T_SPLIT:
                        src, r_off = xa, XA_R0
                    else:
                        src, r_off = xb, XB_R0
                    r0 = RPT * t - r_off
                    for h in range(NH):
                        last = (img == IMGS - 1 and t == NT - 1 and h == NH - 1)
                        # final tile: two halves so the post-matmul drain
                        # (bias-add + out DMA) is half as long
                        parts = 2 if last else 1
                        rows = RPT // parts
                        for j in range(parts):
                            pt = conv_tile(src, r0 + j * rows, h * 128,
                                           rows, rows * W)
                            c0 = t * TF + j * rows * W
                            nc.vector.tensor_scalar_add(
                                out=ots[h][:, c0:c0 + rows * W],
                                in0=pt[:, :rows * W],
                                scalar1=b_sb[:, h:h + 1],
                            )
                            nc.sync.dma_start(
                                out=out_v[img, h * 128:(h + 1) * 128,
                                          c0:c0 + rows * W],
                                in_=ots[h][:, c0:c0 + rows * W])
    nc.compile()
    return nc


def get_program():
    if "nc" not in _CACHE:
        _CACHE["nc"] = _build_program()
    return _CACHE["nc"]


def make_in_maps(x, weight, bias):
    x = np.asarray(x, dtype=np.float32)
    weight = np.asarray(weight, dtype=np.float32)
    bias = np.asarray(bias, dtype=np.float32)

    xpad = np.zeros((N, C_IN, HP, WP), dtype=ml_dtypes.bfloat16)
    xpad[:, :, 1:1 + H, 1:1 + W] = x.astype(ml_dtypes.bfloat16)
    w_t = np.ascontiguousarray(
        weight.transpose(1, 2, 3, 0).reshape(C_IN, KH * KW, C_OUT)
    ).astype(ml_dtypes.bfloat16)
    b2 = np.ascontiguousarray(bias.reshape(NH, 128).T)

    return [
        {
            "xp": np.ascontiguousarray(xpad[i * IMGS:(i + 1) * IMGS]),
            "w": w_t,
            "b": b2,
        }
        for i in range(N_CORES)
    ]


def kernel(x, weight, bias):
    from concourse.bass_utils import run_bass_kernel_spmd

    nc = get_program()
    in_maps = make_in_maps(x, weight, bias)
    res = run_bass_kernel_spmd(nc, in_maps, core_ids=list(range(N_CORES)))
    return np.concatenate([res.results[i]["out"] for i in range(N_CORES)],
                          axis=0)
